# revision 3
# baseline (speedup 1.0000x reference)
"""Trainium2 Bass kernel for nn_FMNet pixel-shuffle + sigmoid.

reference:  x = FV[:, 64:, :, :]                    # [B, 64, 64, 64]
            out[b, 8i+r, 8j+c] = sigmoid(x[b, 8r+c, i, j])   # [B, 1, 512, 512]

Strategy (per core, pure data-parallel over batch, 4 batches/core):
  For each channel-group r in 0..8 (channels 8r..8r+8 of x):
    - one HWDGE load  [4 b x 32 i2 partitions] x [8 c x 128 (ip,j)] -> SBUF
      (contiguous 512-byte chunks in HBM: two spatial rows per partition)
    - one ScalarE activation (Sigmoid) that simultaneously performs the
      (c', j) -> (j*8 + c') interleave via a strided input access pattern
    - one HWDGE store, fully 2 KiB-contiguous rows to the output
  All three streams are double-buffered by construction: 8 independent
  SBUF tile pairs, loads issued back-to-back from SP, compute + stores
  chained on ACT.
"""

import os
import sys

if "/opt/trn_rl_repo" not in sys.path:
    sys.path.insert(0, "/opt/trn_rl_repo")

import numpy as np

import concourse.bass as bass
from concourse import mybir
from concourse.bass_utils import run_bass_kernel_spmd

N_CORES = 8
B = 32          # full batch
B_LOC = B // N_CORES   # 4 batches per core
H = W = 512
S = 64          # spatial dim of x
NG = 8          # channel groups (r)

LAST_EXEC_NS = None

_cached_nc = None


def _install_trace_hook():
    """Best-effort NTFF hook so BASS_TRACE=1 yields exec_time_ns."""
    try:
        import types

        import antenv

        try:
            from antenv.axon_hooks import get_axon_ntff_profile_hook  # noqa: F401

            return
        except ImportError:
            pass
        mod = types.ModuleType("antenv.axon_hooks")
        _state = {"hook": None}
        mod.set_axon_ntff_profile_hook = lambda h: _state.__setitem__("hook", h)
        mod.get_axon_ntff_profile_hook = lambda: _state["hook"]
        sys.modules["antenv.axon_hooks"] = mod
        antenv.axon_hooks = mod
        from trn_agent_boot.trn_boot import _ntff_profile_via_ctypes

        mod.set_axon_ntff_profile_hook(
            _ntff_profile_via_ctypes("/opt/axon/libaxon_pjrt.so")
        )
    except Exception:
        pass


def _build_nc():
    nc = bass.Bass("TRN2", num_devices=N_CORES)
    FV = nc.declare_dram_parameter(
        "FV", [B_LOC, 128, S, S], mybir.dt.float32, isOutput=False
    )
    OUT = nc.declare_dram_parameter(
        "OUT", [B_LOC, W, H], mybir.dt.float32, isOutput=True
    )

    tins = [
        nc.alloc_sbuf_tensor(f"tin{r}", [128, 1024], mybir.dt.float32)
        for r in range(NG)
    ]
    touts = [
        nc.alloc_sbuf_tensor(f"tout{r}", [128, 1024], mybir.dt.float32)
        for r in range(NG)
    ]

    fv = FV[:]
    out = OUT[:]

    import contextlib

    with contextlib.ExitStack() as stack:
        block = stack.enter_context(nc.Block())
        sem_in = [
            stack.enter_context(nc.semaphore(f"sem_in{r}")) for r in range(NG)
        ]
        sem_act = stack.enter_context(nc.semaphore("sem_act"))
        sem_out = stack.enter_context(nc.semaphore("sem_out"))

        @block.sync
        def _(sync: bass.BassEngine):
            for r in range(NG):
                for b in range(B_LOC):
                    # x channels 8r..8r+8 -> [c', i2, (ip j)]; (ip j) is one
                    # contiguous 128-element (512 B) run in HBM.
                    src = fv[b, 64 + 8 * r : 64 + 8 * r + 8]  # [8, 64, 64]
                    src = src.rearrange("c (i2 ip) j -> i2 c (ip j)", ip=2)
                    sync.dma_start(
                        out=tins[r].ap()[32 * b : 32 * b + 32, :], in_=src
                    ).then_inc(sem_in[r], 16)

        @block.scalar
        def _(scalar: bass.BassEngine):
            for r in range(NG):
                scalar.wait_ge(sem_in[r], 16 * B_LOC)
                # strided read (ip, j, c') off the (c', ip, j) tile does the
                # 8-way interleave; write is contiguous (ip, j*8+c')
                tin = tins[r].ap().rearrange("p (c ip j) -> p ip j c", c=8, ip=2)
                scalar.activation(
                    touts[r].ap(), tin, mybir.ActivationFunctionType.Sigmoid
                ).then_inc(sem_act, 1)
                scalar.wait_ge(sem_act, r + 1)
                for b in range(B_LOC):
                    # out rows: 512 = (i2:32, ip:2, r:8); row = 16*i2 + 8*ip + r
                    dst = out[b].rearrange("(i2 ip r) q -> i2 ip r q", i2=32, ip=2)
                    dst = dst[:, :, r, :]  # [32, 2, 512], 2 KiB rows
                    scalar.dma_start(
                        out=dst, in_=touts[r].ap()[32 * b : 32 * b + 32, :]
                    ).then_inc(sem_out, 16)
            scalar.wait_ge(sem_out, 16 * B_LOC * NG)

    return nc


def kernel(FV, batch_size=None, W=None, H=None, **_ignored):
    global _cached_nc, LAST_EXEC_NS
    FV = np.asarray(FV, dtype=np.float32)
    assert FV.shape == (B, 128, S, S), FV.shape

    trace = bool(os.environ.get("BASS_TRACE"))
    if trace:
        _install_trace_hook()

    if _cached_nc is None:
        _cached_nc = _build_nc()
    nc = _cached_nc

    in_maps = [{"FV": FV[k * B_LOC : (k + 1) * B_LOC]} for k in range(N_CORES)]
    res = run_bass_kernel_spmd(nc, in_maps, list(range(N_CORES)), trace=trace)
    if trace:
        LAST_EXEC_NS = res.exec_time_ns

    outs = [res.results[k]["OUT"] for k in range(N_CORES)]
    full = np.concatenate(outs, axis=0)  # [32, 512, 512]
    return full[:, None, :, :].astype(np.float32)


# revision 5
# speedup vs baseline: 1.1834x; 1.1834x over previous
"""Trainium2 Bass kernel for nn_FMNet pixel-shuffle + sigmoid.

reference:  x = FV[:, 64:, :, :]                                 # [B, 64, 64, 64]
            out[b, 8i+r, 8j+c] = sigmoid(x[b, 8r+c, i, j])       # [B, 1, 512, 512]

Per core (4 batches, pure data-parallel over batch):
  - 8 SWDGE loads (gpsimd Q7 generator) of 512 KiB: per (batch, channel-half),
    partition = (b, i2) spatial-row-pair, 512-byte contiguous HBM chunks.
    SWDGE keeps the load descriptor generation off the single shared HWDGE.
  - 8 fused ScalarE ACTIVATE(Sigmoid) ops [128 x 1024] whose strided input AP
    performs the (c', j) -> (j*8 + c') pixel-shuffle interleave in the same
    pass (measured ~2 ns/elem; DVE/GpSimd strided copies are ~4.4 ns/elem).
  - 16 HWDGE stores (SP engine, now otherwise idle) of 256 KiB: per
    (batch, r-quarter), 4 KiB contiguous HBM chunks, issued as soon as the
    two ACTs they depend on are done - keeps the store tail short.
"""

import os
import sys

if "/opt/trn_rl_repo" not in sys.path:
    sys.path.insert(0, "/opt/trn_rl_repo")

import numpy as np

import concourse.bass as bass
from concourse import mybir
from concourse.bass_utils import run_bass_kernel_spmd

N_CORES = 8
B = 32
B_LOC = B // N_CORES   # 4
H = W = 512
S = 64
NG = 8                 # channel groups (r)

LAST_EXEC_NS = None

_cached_nc = None


def _install_trace_hook():
    """Best-effort NTFF hook so BASS_TRACE=1 yields exec_time_ns."""
    try:
        import types

        import antenv

        try:
            from antenv.axon_hooks import get_axon_ntff_profile_hook  # noqa: F401

            return
        except ImportError:
            pass
        mod = types.ModuleType("antenv.axon_hooks")
        _state = {"hook": None}
        mod.set_axon_ntff_profile_hook = lambda h: _state.__setitem__("hook", h)
        mod.get_axon_ntff_profile_hook = lambda: _state["hook"]
        sys.modules["antenv.axon_hooks"] = mod
        antenv.axon_hooks = mod
        from trn_agent_boot.trn_boot import _ntff_profile_via_ctypes

        mod.set_axon_ntff_profile_hook(
            _ntff_profile_via_ctypes("/opt/axon/libaxon_pjrt.so")
        )
    except Exception:
        pass


def _build_nc():
    import contextlib

    F32 = mybir.dt.float32
    nc = bass.Bass("TRN2", num_devices=N_CORES)
    FV = nc.declare_dram_parameter("FV", [B_LOC, 128, S, S], F32, isOutput=False)
    OUT = nc.declare_dram_parameter("OUT", [B_LOC, W, H], F32, isOutput=True)

    # TIN_h[p=(b,i2), (c32, ip, j)] for channel half h (c32 = c' within half)
    tin = [nc.alloc_sbuf_tensor(f"tin{h}", [128, 4096], F32) for h in range(2)]
    # TOUT_h[p=(b,i2), (ip, r4, q)] for r half h
    tout = [nc.alloc_sbuf_tensor(f"tout{h}", [128, 4096], F32) for h in range(2)]

    fv = FV[:]
    out = OUT[:]

    with contextlib.ExitStack() as stack:
        block = stack.enter_context(nc.Block())
        sem_in = [stack.enter_context(nc.semaphore(f"sem_in{h}")) for h in range(2)]
        sem_act = stack.enter_context(nc.semaphore("sem_act"))
        sem_out = stack.enter_context(nc.semaphore("sem_out"))

        @block.gpsimd
        def _(g: bass.BassEngine):
            for h in range(2):
                for b in range(B_LOC):
                    # channels [64+32h, 64+32h+32): [c32, i2, (ip j)]
                    src = fv[b, 64 + 32 * h : 64 + 32 * h + 32]  # [32, 64, 64]
                    src = src.rearrange("c (i2 ip) j -> i2 c (ip j)", ip=2)
                    g.dma_start(
                        out=tin[h].ap()[32 * b : 32 * b + 32, :], in_=src
                    ).then_inc(sem_in[h], 16)

        @block.scalar
        def _(scalar: bass.BassEngine):
            for r in range(NG):
                h, r4 = divmod(r, 4)
                if r4 == 0:
                    scalar.wait_ge(sem_in[h], 16 * B_LOC)
                # in: (ip, j, c') strided read of the (c', ip, j) tile slice
                tin_v = (
                    tin[h]
                    .ap()[:, 1024 * r4 : 1024 * (r4 + 1)]
                    .rearrange("p (c ip j) -> p ip j c", c=8, ip=2)
                )
                # out: (ip, [r4], q) with q = j*8+c' contiguous
                tout_v = tout[h].ap().rearrange(
                    "p (ip r4 q) -> p ip r4 q", ip=2, r4=4
                )[:, :, r4, :]
                scalar.activation(
                    tout_v, tin_v, mybir.ActivationFunctionType.Sigmoid
                ).then_inc(sem_act, 1)

        @block.sync
        def _(sync: bass.BassEngine):
            for rq in range(4):          # r-quarter: r in {2rq, 2rq+1}
                h, k = divmod(rq, 2)     # tout half h, quarter k within half
                sync.wait_ge(sem_act, 2 * (rq + 1))
                for b in range(B_LOC):
                    # dest rows 16*i2 + 8*ip + (2rq + r2), cols q
                    dst = out[b].rearrange(
                        "(i2 ip rr r2) q -> i2 ip rr (r2 q)", i2=32, ip=2, rr=4
                    )[:, :, rq, :]  # [32, 2, 1024]
                    src = tout[h].ap().rearrange(
                        "p (ip r2 v) -> p ip r2 v", ip=2, r2=2
                    )[32 * b : 32 * b + 32, :, k, :]  # [32, 2, 1024]
                    sync.dma_start(out=dst, in_=src).then_inc(sem_out, 16)
            sync.wait_ge(sem_out, 16 * 4 * B_LOC)

    return nc


def kernel(FV, batch_size=None, W=None, H=None, **_ignored):
    global _cached_nc, LAST_EXEC_NS
    FV = np.asarray(FV, dtype=np.float32)
    assert FV.shape == (B, 128, S, S), FV.shape

    trace = bool(os.environ.get("BASS_TRACE"))
    if trace:
        _install_trace_hook()

    if _cached_nc is None:
        _cached_nc = _build_nc()
    nc = _cached_nc

    in_maps = [{"FV": FV[k * B_LOC : (k + 1) * B_LOC]} for k in range(N_CORES)]
    res = run_bass_kernel_spmd(nc, in_maps, list(range(N_CORES)), trace=trace)
    if trace:
        LAST_EXEC_NS = res.exec_time_ns

    outs = [res.results[k]["OUT"] for k in range(N_CORES)]
    full = np.concatenate(outs, axis=0)  # [32, 512, 512]
    return full[:, None, :, :].astype(np.float32)
